# revision 9
# baseline (speedup 1.0000x reference)
# Trainium2 Bass kernel for: ConvTranspose2d(64->128, k=4, stride=1) -> spatial
# mean -> +biases -> 10*logsumexp over channels.
#
# Math: with full (K-1) output padding, the mean over the ENTIRE conv-transpose
# output spatial extent sees every input pixel through all K*K taps, so
#   pooled[n,co] = (sum_hw x[n,ci,hw]) @ (sum_kk w[ci,co,kk]) / (Ho*Wo) + cb + eb
# exactly. The conv collapses to a spatial sum + a (Cin x Cout) matmul.
#
# Sharding: data-parallel over batch N=32 across 8 cores (4 batches/core),
# params replicated.
#
# v3 (from v2 trace analysis; v1 31.65us, v2 28.88us):
# - x ships bf16 (2.25 MiB/core stream, rel err ~1e-4 vs the 2e-2 gate).
# - NO in-flight DMA throttle (bufs=NCK): gating chunk issues on reduce
#   completion (v2 bufs=4, v3 bufs=3) collapses aggregate DMA bandwidth to
#   150-180 GB/s: queued transfers share SDMA round-robin and complete at
#   the END, so the issue->latency->reduce->free loop paces chunks at ~2us.
#   Unthrottled, the Sync engine's own ~0.7us/issue staggers queue entry, so
#   completions stagger too and the stream runs at full aggregate BW (v1
#   measured 328 GB/s this way) while reduces chase completions.
# - Chunk spatial sums rotate across THREE engines (DVE tensor_scalar+accum
#   1.22us, ACT Copy+accum 1.43us incl accumulator read, GpSimd
#   tensor_scalar+accum) so reduce cadence beats the 0.78us/chunk transfer
#   cadence. All accumulate fp32.
# - Weight k-sum runs on DVE over a host-packed [128, 1024] layout (k-halves
#   stacked on partitions, halving 1x-mode reduce cycles); the half-sum +
#   partition-duplication G -> wdup runs as a PE matmul against a host-built
#   0/1 dup matrix D (D[p,m] = p%64==m%64), PSUM -> SBUF copy on ACT. PE is
#   otherwise idle mid-stream.
# - Manual LoadActFuncSet("natural_log_exp_and_others" = Copy+Exp+Ln) must be
#   the FIRST Scalar instruction: emitted after any scalar DMA, the
#   insert_act_table_loads pass adds a spurious second set-load (v2 paid
#   2x 1.28us ACT_TABLE_LOAD).
# - Tail: pooled computed TRANSPOSED [co, n] so the 1/(Ho*Wo) scale and both
#   biases fold into the Exp activation (func(scale*in+bias), bias = fp32
#   per-partition AP on the co axis); channel sum = tiny PE matmul against a
#   ones vector; Ln reads PSUM; x10 on DVE; 16-byte result DMA.

import os

import numpy as np
import ml_dtypes

import concourse.bacc as bacc
import concourse.bass as bass
import concourse.mybir as mybir
import concourse.tile as tile
from concourse.bass_utils import run_bass_kernel_spmd
from concourse.hw_specs import get_activation_tables

N, CIN, COUT, K, H, W = 32, 64, 128, 4, 64, 64
NCORES = 8
NLOC = N // NCORES          # 4 batches per core
HW = H * W                  # 4096
ROWS = NLOC * CIN           # 256 rows (n,ci) per core
RBLK = ROWS // 128          # 2 row blocks of 128 partitions
CHUNKS = [2048, 1536, 512]  # per-block column chunk widths: big chunks first
                            # (4KB/partition lines, best DMA efficiency), small
                            # last chunk so the final reduce trail is short
NCHUNK = len(CHUNKS)
NCK = RBLK * NCHUNK         # 6 chunks total
SCALE = 1.0 / float((H + K - 1) * (W + K - 1))   # 1/4489
KHALF = K * K // 2          # 8 kernel taps per partition-half in wk packing

F32 = mybir.dt.float32
BF16 = mybir.dt.bfloat16
BF16_NP = ml_dtypes.bfloat16

# issue-order engine map: V=DVE, A=ACT. (GpSimd rejected: the Pool engine
# has no TensorScalarPtr-reduce.) Issue order interleaves blocks:
# (b0,2048)V (b1,2048)A (b0,1536)A (b1,1536)V (b0,512)A (b1,512)V.
ENGINE_OF = ['V', 'A', 'A', 'V', 'A', 'V']

_CACHE: dict = {}


def _build_module() -> bacc.Bacc:
    nc = bacc.Bacc("TRN2", target_bir_lowering=False, enable_partition_id=False)

    x_d = nc.dram_tensor("xc", [ROWS, HW], BF16, kind="ExternalInput").ap()
    w_d = nc.dram_tensor("w", [128, COUT * KHALF + 128], BF16, kind="ExternalInput").ap()
    bs_d = nc.dram_tensor("bs", [COUT, 2], F32, kind="ExternalInput").ap()
    y_d = nc.dram_tensor("y", [1, NLOC], F32, kind="ExternalOutput").ap()

    with tile.TileContext(nc) as tc:
        with (
            tc.tile_pool(name="xpool", bufs=RBLK) as xpool,
            tc.tile_pool(name="small", bufs=1) as small,
            tc.tile_pool(name="psw", bufs=1, space="PSUM") as psw_pool,
            tc.tile_pool(name="ps2", bufs=1, space="PSUM") as ps2_pool,
            tc.tile_pool(name="ps3", bufs=1, space="PSUM") as ps3_pool,
        ):
            # ---- ACT table preload MUST be the first Scalar instruction ----
            act_tables = get_activation_tables(nc.m.arch)
            set_id = next(
                i
                for i, (_, funcs) in enumerate(act_tables.items())
                if mybir.ActivationFunctionType.Exp in funcs
                and mybir.ActivationFunctionType.Ln in funcs
                and mybir.ActivationFunctionType.Copy in funcs
            )
            nc.scalar.add_instruction(
                mybir.InstLoadActFuncSet(
                    name=nc.get_next_instruction_name(), act_func_set_id=set_id
                )
            )

            # ---- param DMAs on the ACT HWDGE ring (x rides the SP ring).
            # wk and the dup matrix ride ONE DMA: the Tile scheduler has only
            # 8 DMA-completion sem lanes and v4's 11 DMAs made late x-chunk
            # issues wait on earlier chunks' completions. ----
            wd_t = small.tile([128, COUT * KHALF + 128], BF16)
            nc.scalar.dma_start(out=wd_t, in_=w_d)
            wk = wd_t[:, 0 : COUT * KHALF]
            dmat = wd_t[:, COUT * KHALF : COUT * KHALF + 128]
            bsrows = small.tile([COUT, 2], F32)
            nc.scalar.dma_start(out=bsrows, in_=bs_d)

            # ---- small constants / scratch (DVE, early) ----
            s2m = small.tile([128, NLOC], BF16)
            nc.vector.memset(s2m, 0.0)
            onesb = small.tile([128, 1], BF16)
            nc.vector.memset(onesb, 1.0)
            biasc = small.tile([COUT, 1], F32)
            nc.vector.reduce_sum(out=biasc, in_=bsrows, axis=mybir.AxisListType.X)

            # ---- spatial sums of x on a 3-engine rotation; weight k-half
            # sums (G) interleaved on DVE ----
            parts = small.tile([128, NCK], F32)
            gsum = small.tile([128, COUT], F32)
            gsum_b = small.tile([128, COUT], BF16)
            scrV = small.tile([128, max(CHUNKS)], BF16)
            scrA = small.tile([128, max(CHUNKS)], BF16)

            wkv = wk.rearrange("p (c k) -> p c k", k=KHALF)
            WPIECE = COUT // 2  # 64 output channels per weight-reduce piece
            wpieces_done = 0

            col0 = [0]
            for w_ in CHUNKS[:-1]:
                col0.append(col0[-1] + w_)
            issue_order = [(r, c) for c in range(NCHUNK) for r in range(RBLK)]
            for idx, (r, c) in enumerate(issue_order):
                cw = CHUNKS[c]
                xt = xpool.tile([128, cw], BF16, tag=f"xt{c}", bufs=RBLK)
                nc.sync.dma_start(
                    out=xt,
                    in_=x_d[r * 128 : (r + 1) * 128, col0[c] : col0[c] + cw],
                )
                pcol = parts[:, r * NCHUNK + c : r * NCHUNK + c + 1]
                eng = ENGINE_OF[idx]
                if eng == 'V':
                    nc.vector.tensor_scalar(
                        out=scrV[:, 0:cw],
                        in0=xt,
                        scalar1=0.0,
                        scalar2=None,
                        op0=mybir.AluOpType.add,
                        op1=mybir.AluOpType.add,
                        accum_out=pcol,
                    )
                    if wpieces_done < 2:
                        j = wpieces_done
                        nc.vector.reduce_sum(
                            out=gsum[:, j * WPIECE : (j + 1) * WPIECE],
                            in_=wkv[:, j * WPIECE : (j + 1) * WPIECE, :],
                            axis=mybir.AxisListType.X,
                        )
                        wpieces_done += 1
                        if wpieces_done == 2:
                            nc.vector.tensor_copy(gsum_b, gsum)
                elif eng == 'A':
                    nc.scalar.activation(
                        out=scrA[:, 0:cw],
                        in_=xt,
                        func=mybir.ActivationFunctionType.Copy,
                        accum_out=pcol,
                    )

            # ---- wdup[m,co] = G[m%64,co] + G[64+m%64,co] via PE dup-matmul,
            # mid-stream (PE idle); PSUM -> SBUF bf16 copy on ACT ----
            psw = psw_pool.tile([128, COUT], F32, space="PSUM")
            nc.tensor.matmul(out=psw, lhsT=dmat, rhs=gsum_b, start=True, stop=True)
            wdup = small.tile([128, COUT], BF16)
            nc.scalar.activation(
                out=wdup, in_=psw, func=mybir.ActivationFunctionType.Copy
            )

            # ---- combine chunk partials: s2[p, r] = spatial sum of block r
            s2 = small.tile([128, RBLK], F32)
            nc.vector.reduce_sum(
                out=s2,
                in_=parts.rearrange("p (r c) -> p r c", r=RBLK),
                axis=mybir.AxisListType.X,
            )

            # ---- masked rhs (128, 4) bf16: col n nonzero only on its own
            # partition half: s2m[(n%2)*64 + ci, n] = S[n, ci]
            s2m_v = s2m.rearrange("p (r t) -> p r t", t=2)
            s2_v = s2.rearrange("p (r t) -> p r t", t=1)
            nc.vector.tensor_copy(s2m_v[0:64, :, 0:1], s2_v[0:64, :, :])
            nc.vector.tensor_copy(s2m_v[64:128, :, 1:2], s2_v[64:128, :, :])

            # ---- pooled^T (co, n) in PSUM via one bf16 matmul ----
            pooledT = ps2_pool.tile([COUT, NLOC], F32, space="PSUM")
            nc.tensor.matmul(out=pooledT, lhsT=wdup, rhs=s2m, start=True, stop=True)

            # ---- exp(SCALE * pooledT + (cb+eb)[co]) -> bf16 SBUF ----
            expT = small.tile([COUT, NLOC], BF16)
            nc.scalar.activation(
                out=expT,
                in_=pooledT,
                func=mybir.ActivationFunctionType.Exp,
                bias=biasc,
                scale=float(SCALE),
            )

            # ---- sum over channels (partition axis) via PE against ones ----
            sume = ps3_pool.tile([1, NLOC], F32, space="PSUM")
            nc.tensor.matmul(out=sume, lhsT=onesb, rhs=expT, start=True, stop=True)

            # ---- 10 * ln(sum) ----
            logv = small.tile([1, NLOC], F32)
            nc.scalar.activation(
                out=logv, in_=sume, func=mybir.ActivationFunctionType.Ln
            )
            outv = small.tile([1, NLOC], F32)
            nc.vector.tensor_scalar_mul(out=outv, in0=logv, scalar1=10.0)
            nc.sync.dma_start(out=y_d, in_=outv)

    nc.compile()
    return nc


def kernel(x, weight, conv_bias, extra_bias):
    x = np.asarray(x, dtype=np.float32)
    weight = np.asarray(weight, dtype=np.float32)
    conv_bias = np.asarray(conv_bias, dtype=np.float32)
    extra_bias = np.asarray(extra_bias, dtype=np.float32)
    assert x.shape == (N, CIN, H, W), x.shape
    assert weight.shape == (CIN, COUT, K, K), weight.shape

    if "nc" not in _CACHE:
        _CACHE["nc"] = _build_module()
    nc = _CACHE["nc"]

    # weight packed [h*64+ci, co*8+k']: G[h*64+ci, co] sums taps k = 8h+k'
    w7 = (
        weight.reshape(CIN, COUT, 2, KHALF)
        .transpose(2, 0, 1, 3)
        .reshape(128, COUT * KHALF)
        .astype(BF16_NP)
    )
    dmat = (
        np.arange(128)[:, None] % 64 == np.arange(128)[None, :] % 64
    ).astype(BF16_NP)
    wd = np.ascontiguousarray(np.concatenate([w7, dmat], axis=1))
    bs2 = np.ascontiguousarray(
        np.stack([conv_bias, extra_bias], axis=1).astype(np.float32)
    )  # (COUT, 2)
    xb = x.astype(BF16_NP)
    in_maps = []
    for c in range(NCORES):
        xc = np.ascontiguousarray(xb[c * NLOC : (c + 1) * NLOC].reshape(ROWS, HW))
        in_maps.append({"xc": xc, "w": wd, "bs": bs2})

    trace = os.environ.get("BASS_KERNEL_TRACE") == "1"
    res = run_bass_kernel_spmd(
        nc, in_maps, core_ids=list(range(NCORES)), trace=trace
    )
    _CACHE["last_result"] = res
    # each core returns y[1, NLOC]; stack -> (NCORES, NLOC) -> (N, 1)
    return np.concatenate([r["y"] for r in res.results], axis=0).reshape(N, 1)
